# revision 32
# baseline (speedup 1.0000x reference)
"""Angular LSH bucketing kernel for 8 TRN2 NeuronCores.

Reference computation:
    scores  = mat @ proj_dir          # [b, h, n, 8]
    bits    = scores > 0
    bin_ids = sum(bits * 2^r)
    buckets = perm[bin_ids]           # perm is the Gray-code table

Sharding: data-parallel over batch*heads (64 -> 8 per core); proj/perm/enc
replicated (tiny).

Device strategy (per core, 65536 rows of 64 f32):
  - Host splits each f32 into hi+lo bf16 (a + b), viewed as [pairs, 128]
    bf16 (two consecutive 64-dim rows = one 128-deep column).
  - X-bar DMA-transpose loads [128, pairs] tiles at ~full HBM bandwidth.
  - 3 bf16 matmuls per 128-pair slice accumulate fp32 scores in PSUM
    (A*p_hi + A*p_lo + B*p_hi; B*p_lo term is below fp32 noise), using a
    block-diagonal projection so even/odd rows both get all 8 projections:
    out psum [128 pairs, (parity, proj)] -- rows on partitions.
  - Vector stage (wide [128, 512] ops): bits = scores > 0; for the Gray
    permutation perm[x] = x ^ (x >> 1), bucket bit r = bits_r XOR
    bits_{r+1} (bit 7 = bits_7), so bucket = sum_r (bits_r != bits_{r+1})
    * 2^r with the r=7 slot patched to bits_7 * 128. Grouped 8-wide
    reduce, cast to int32, DMA out.
  - If perm/enc_vec are not the Gray-code/power-of-two tables, falls back
    to computing bin_ids on device and applying perm on the host.
"""

import numpy as np
import ml_dtypes

from concourse import bass, mybir, tile
from concourse.bass_utils import run_bass_kernel_spmd

N_CORES = 8
B, H, N, D = 2, 32, 8192, 64
NPROJ = 8
ROWS_PER_CORE = (B * H // N_CORES) * N  # 65536
PAIRS = ROWS_PER_CORE // 2  # 32768
CHUNK_PAIRS = 4096
NCHUNK = PAIRS // CHUNK_PAIRS  # 8
U_PER_CHUNK = CHUNK_PAIRS // 128  # 32

F32 = mybir.dt.float32
BF16 = mybir.dt.bfloat16
I32 = mybir.dt.int32

_cache = {}



TAU = 0.08  # |score| threshold below which the host recomputes the row exactly
# (bf16 mat AND bf16 proj: score err std ~0.013, TAU ~ 6 sigma)


def _build_v3(gray: bool, pairs: int = PAIRS):
    """Natural full-bandwidth loads + PE transpose + ACT evacuation.

    Input is a single bf16 image of mat ("halves" pairing: column block j
    holds rows q + j*PAIRS), loaded contiguously at full HBM bandwidth.
    The X-bar transpose path is avoided entirely (measured ~115 GB/s packet
    ceiling); instead TensorE transposes [128,128] tiles through PSUM and
    ScalarE evacuates them. A per-row min|score| is emitted so the caller
    can recompute rows where bf16 rounding could flip a sign bit.
    """
    n_r = pairs // 128          # transpose tiles (r values)
    nchunk = n_r // 32          # DMA/psum chunks of 32 r each
    ngroups = n_r // 4          # transpose/evac groups of 4 r each
    fw = 512                    # psum free width per chunk (32 r * 16)
    nc = bass.Bass()
    a_d = nc.declare_dram_parameter("a", [pairs, 128], BF16, isOutput=False)
    pw_d = nc.declare_dram_parameter("pw", [128, 32], BF16, isOutput=False)
    w_d = nc.declare_dram_parameter("w", [128, fw], BF16, isOutput=False)
    id_d = nc.declare_dram_parameter("ident", [128, 128], BF16, isOutput=False)
    out_d = nc.declare_dram_parameter("out", [2 * pairs], I32, isOutput=True)
    ma_d = nc.declare_dram_parameter("ma", [2 * pairs], F32, isOutput=True)

    from contextlib import ExitStack

    with ExitStack() as ctx:
        ent = ctx.enter_context
        a_n = ent(nc.sbuf_tensor("a_n", [128, n_r, 128], BF16))
        pw_sb = ent(nc.sbuf_tensor("pw_sb", [128, 32], BF16))
        w_sb = ent(nc.sbuf_tensor("w_sb", [128, fw], BF16))
        id_sb = ent(nc.sbuf_tensor("id_sb", [128, 128], BF16))
        mt_sb = ent(nc.sbuf_tensor("mt_sb", [128, 4, 512], BF16))  # 4 group slots
        bt = ent(nc.sbuf_tensor("bt", [128, fw], F32))
        g = ent(nc.sbuf_tensor("g", [128, fw], F32))
        m = ent(nc.sbuf_tensor("m", [128, fw], F32))
        bf = ent(nc.sbuf_tensor("bf", [128, fw // 8], F32))
        fence_sb = ent(nc.sbuf_tensor("fence_sb", [128, 2], BF16))
        bi = ent(nc.sbuf_tensor("bi", [128, nchunk * fw // 8], I32))
        ma_sb = ent(nc.sbuf_tensor("ma_sb", [128, nchunk * fw // 8], F32))
        ps = ent(nc.psum_tensor("ps", [128, 3, fw], F32))       # 3 chunk slots
        scr = ent(nc.psum_tensor("scr", [128, 128], F32))       # fence scratch
        pst = ent(nc.psum_tensor("pst", [128, 4, 1024], BF16))  # 4 group slots, one bank each

        cs_sem = ent(nc.semaphore("cs_sem"))
        ch_sems = [ent(nc.semaphore(f"ch_sem{c}")) for c in range(nchunk)]
        pet_sem = ent(nc.semaphore("pet_sem"))    # transpose groups done (PE)
        act_sem = ent(nc.semaphore("act_sem"))    # evac groups done (ACT)
        pemm_sem = ent(nc.semaphore("pemm_sem"))  # MM groups done (PE)
        dve_sem = ent(nc.semaphore("dve_sem"))    # chunks done (DVE)
        dvrel_sem = ent(nc.semaphore("dvrel_sem"))  # chunk fence (PE dummy MM)
        out_sem = ent(nc.semaphore("out_sem"))

        def mm_group(tensor, gg):
            k = gg // 8
            if gg % 8 == 0 and k >= 3:
                tensor.wait_ge(dve_sem, k - 2)  # psum chunk slot reuse
            for i in range(4):
                r = 4 * gg + i
                ri = r % 32
                lhsT = mt_sb[:, gg % 4, 128 * i : 128 * (i + 1)]
                o = ps[:, k % 3, 16 * ri : 16 * (ri + 1)]
                tensor.matmul(o, lhsT, pw_sb[:, 0:16], start=True, stop=False)
                mm = tensor.matmul(
                    o, lhsT, pw_sb[:, 16:32], start=False, stop=True
                )
            mm.then_inc(pemm_sem, 1)
            if gg % 8 == 7:
                # fence: a dummy matmul whose 128-column fill outlasts the
                # prior matmul's PSUM drain; its inc releases the DVE read.
                tensor.matmul(
                    scr[0:16, :], pw_sb[:, 0:16], id_sb[:], start=True, stop=True
                ).then_inc(dvrel_sem, 1)

        with nc.Block() as block:

            @block.sync
            def _(sync):
                sync.dma_start(out=pw_sb[:], in_=pw_d[:]).then_inc(cs_sem, 16)
                sync.dma_start(out=w_sb[:], in_=w_d[:]).then_inc(cs_sem, 16)
                sync.dma_start(out=id_sb[:], in_=id_d[:]).then_inc(cs_sem, 16)
                a_view = a_d[:].rearrange("(P r) c -> P r c", P=128)
                for k in range(nchunk):
                    sync.dma_start(
                        out=a_n[:, 32 * k : 32 * (k + 1), :],
                        in_=a_view[:, 32 * k : 32 * (k + 1), :],
                    ).then_inc(ch_sems[k], 16)
                sync.wait_ge(dve_sem, nchunk)
                for k in range(nchunk):
                    csl = slice(k * fw // 8, (k + 1) * fw // 8)
                    dst = out_d[:].rearrange(
                        "(j P kk ri) -> P j kk ri", j=2, P=128, kk=nchunk
                    )[:, :, k, :]
                    sync.dma_start(
                        out=dst,
                        in_=bi[:, csl].rearrange("p (j ri) -> p j ri", j=2),
                    ).then_inc(out_sem, 16)
                    dst2 = ma_d[:].rearrange(
                        "(j P kk ri) -> P j kk ri", j=2, P=128, kk=nchunk
                    )[:, :, k, :]
                    sync.dma_start(
                        out=dst2,
                        in_=ma_sb[:, csl].rearrange("p (j ri) -> p j ri", j=2),
                    ).then_inc(out_sem, 16)
                sync.wait_ge(out_sem, 32 * nchunk)

            @block.tensor
            def _(tensor):
                tensor.wait_ge(cs_sem, 48)

                def t_group(gg):
                    k = gg // 8
                    if gg % 8 == 0:
                        tensor.wait_ge(ch_sems[k], 16)
                    for i in range(4):
                        r = 4 * gg + i
                        t = tensor.transpose(
                            pst[:, gg % 4, 128 * i : 128 * (i + 1)],
                            a_n[:, r, :],
                            id_sb[:],
                        )
                    t.then_inc(pet_sem, 4)

                # transposes run two groups ahead of the matmuls so the
                # scalar-engine evacuation pipelines instead of ping-ponging
                t_group(0)
                t_group(1)
                t_group(2)
                for gg in range(3, ngroups):
                    t_group(gg)
                    tensor.wait_ge(act_sem, gg - 2)
                    mm_group(tensor, gg - 3)
                for gg in range(ngroups - 3, ngroups):
                    tensor.wait_ge(act_sem, gg + 1)
                    mm_group(tensor, gg)

            @block.scalar
            def _(scalar):
                for gp in range(ngroups // 2):
                    g0 = 2 * gp  # evacuate groups g0, g0+1 in one pass
                    scalar.wait_ge(pet_sem, 4 * (g0 + 2))
                    if g0 >= 4:
                        scalar.wait_ge(pemm_sem, g0 - 2)
                    scalar.copy(
                        mt_sb[:, g0 % 4 : g0 % 4 + 2, :],
                        pst[:, g0 % 4 : g0 % 4 + 2, 0:512],
                    )
                    # fence: re-read the tail of the copy on the same engine
                    # so the inc cannot outrun the final SBUF writes
                    scalar.copy(
                        fence_sb[:], mt_sb[:, g0 % 4 + 1, 510:512]
                    ).then_inc(act_sem, 2)

            @block.vector
            def _(vector):
                vector.wait_ge(cs_sem, 48)
                for k in range(nchunk):
                    vector.wait_ge(dvrel_sem, k + 1)
                    psl = ps[:, k % 3, :]
                    vector.tensor_single_scalar(
                        bt[:], psl, 0.0, mybir.AluOpType.is_gt
                    )
                    if gray:
                        vector.tensor_tensor(
                            g[:, 0 : fw - 1],
                            bt[:, 0 : fw - 1],
                            bt[:, 1:fw],
                            op=mybir.AluOpType.not_equal,
                        )
                        vector.tensor_copy(g[:, 7:fw:8], bt[:, 7:fw:8])
                        msrc = g
                    else:
                        msrc = bt
                    vector.tensor_mul(m[:], msrc[:], w_sb[:])
                    csl = slice(k * fw // 8, (k + 1) * fw // 8)
                    vector.tensor_reduce(
                        bf[:].rearrange("p (j u) -> p u j", u=32),
                        m[:].rearrange("p (u r) -> p u r", r=8),
                        axis=mybir.AxisListType.X,
                        op=mybir.AluOpType.add,
                    )
                    vector.tensor_copy(bi[:, csl], bf[:])
                    vector.tensor_reduce(
                        ma_sb[:, csl].rearrange("p (j u) -> p u j", u=32),
                        psl.rearrange("p (u r) -> p u r", r=8),
                        axis=mybir.AxisListType.X,
                        op=mybir.AluOpType.min,
                        apply_absolute_value=True,
                    ).then_inc(dve_sem, 1)
    return nc


def _prep_v3(mat, proj_dir, enc_vec, gray):
    bf16 = ml_dtypes.bfloat16
    flat = np.ascontiguousarray(mat.reshape(B * H * N, D), dtype=np.float32)
    rows_per_core = ROWS_PER_CORE
    p = np.asarray(proj_dir, dtype=np.float32).reshape(D, NPROJ)
    pa = p.astype(bf16)
    pb = (p - pa.astype(np.float32)).astype(bf16)
    pw = np.zeros((128, 32), dtype=bf16)
    pw[0:64, 0:8] = pa
    pw[64:128, 8:16] = pa
    pw[0:64, 16:24] = pb
    pw[64:128, 24:32] = pb
    wrow = (
        np.array([1, 2, 4, 8, 16, 32, 64, 128], dtype=np.float32)
        if gray
        else np.asarray(enc_vec, dtype=np.float32).reshape(NPROJ)
    )
    w = np.tile(wrow, 64).reshape(1, 512).repeat(128, axis=0).astype(bf16)
    ident = np.eye(128, dtype=bf16)

    a16 = flat.astype(bf16)
    in_maps = []
    for i in range(N_CORES):
        sh = a16[i * rows_per_core : (i + 1) * rows_per_core]
        a = np.concatenate([sh[:PAIRS], sh[PAIRS:]], axis=1)  # [PAIRS, 128]
        in_maps.append(
            {
                "a": np.ascontiguousarray(a),
                "pw": pw,
                "w": np.ascontiguousarray(w),
                "ident": ident,
            }
        )
    return in_maps


def _build_raw(gray: bool, pairs: int = PAIRS, chunk_pairs: int = CHUNK_PAIRS):
    """Raw-bass (no Tile) build: explicit per-engine streams + semaphores.

    The nix-packaged walrus accepts at most ONE sync wait per instruction
    and has no legalizer pass, which rules out TileContext (its tail drain
    always carries one wait per active processor). Raw streams let every
    cross-engine edge be a standalone wait_ge.
    """
    nchunk = pairs // chunk_pairs
    u_per_chunk = chunk_pairs // 128
    fw = 16 * u_per_chunk  # psum/free width per chunk (512 for default)
    assert nchunk * fw <= 4096, "psum overflow"
    nc = bass.Bass()
    a_d = nc.declare_dram_parameter("a", [pairs, 128], BF16, isOutput=False)
    b_d = nc.declare_dram_parameter("b", [pairs, 128], BF16, isOutput=False)
    pw_d = nc.declare_dram_parameter("pw", [32, 128], BF16, isOutput=False)
    w_d = nc.declare_dram_parameter("w", [fw, 128], BF16, isOutput=False)
    out_d = nc.declare_dram_parameter(
        "out", [nchunk, 2 * chunk_pairs], I32, isOutput=True
    )

    from contextlib import ExitStack

    with ExitStack() as ctx:
        ent = ctx.enter_context
        a_sb = ent(nc.sbuf_tensor("a_sb", [128, pairs], BF16))
        b_sb = ent(nc.sbuf_tensor("b_sb", [128, pairs], BF16))
        pw_sb = ent(nc.sbuf_tensor("pw_sb", [128, 32], BF16))
        w_sb = ent(nc.sbuf_tensor("w_sb", [128, fw], BF16))
        bt = ent(nc.sbuf_tensor("bt", [128, fw], F32))
        g = ent(nc.sbuf_tensor("g", [128, fw], F32))
        m = ent(nc.sbuf_tensor("m", [128, fw], F32))
        bf = ent(nc.sbuf_tensor("bf", [128, fw // 8], F32))
        fence_sb = ent(nc.sbuf_tensor("fence_sb", [128, 2], BF16))
        bi = ent(nc.sbuf_tensor("bi", [128, nchunk * fw // 8], I32))
        ps = ent(nc.psum_tensor("ps", [128, nchunk * fw], F32))

        cs_sem = ent(nc.semaphore("cs_sem"))
        ch_sems = [ent(nc.semaphore(f"ch_sem{c}")) for c in range(nchunk)]
        mm_sem = ent(nc.semaphore("mm_sem"))
        dve_sem = ent(nc.semaphore("dve_sem"))
        out_sem = ent(nc.semaphore("out_sem"))

        with nc.Block() as block:

            @block.sync
            def _(sync):
                sync.dma_start(
                    out=pw_sb[:], in_=pw_d[:], transpose=True
                ).then_inc(cs_sem, 16)
                sync.dma_start(
                    out=w_sb[:], in_=w_d[:], transpose=True
                ).then_inc(cs_sem, 16)
                for c in range(nchunk):
                    sl = slice(c * chunk_pairs, (c + 1) * chunk_pairs)
                    sync.dma_start(
                        out=a_sb[:, sl], in_=a_d[sl, :], transpose=True
                    ).then_inc(ch_sems[c], 16)
                    sync.dma_start(
                        out=b_sb[:, sl], in_=b_d[sl, :], transpose=True
                    ).then_inc(ch_sems[c], 16)
                # outputs: only after ALL compute (also keeps the xbar
                # transpose mode quiesced before normal DMAs run)
                sync.wait_ge(dve_sem, nchunk)
                for c in range(nchunk):
                    sync.dma_start(
                        out=out_d[c].rearrange("(u p j) -> p u j", p=128, j=2),
                        in_=bi[:, c * fw // 8 : (c + 1) * fw // 8].rearrange(
                            "p (u j) -> p u j", j=2
                        ),
                    ).then_inc(out_sem, 16)
                sync.wait_ge(out_sem, 16 * nchunk)

            @block.tensor
            def _(tensor):
                tensor.wait_ge(cs_sem, 32)
                for c in range(nchunk):
                    tensor.wait_ge(ch_sems[c], 32)
                    for u in range(u_per_chunk):
                        col = c * chunk_pairs + u * 128
                        lhsA = a_sb[:, col : col + 128]
                        lhsB = b_sb[:, col : col + 128]
                        o = ps[:, c * fw + u * 16 : c * fw + (u + 1) * 16]
                        tensor.matmul(
                            o, lhsA, pw_sb[:, 0:16], start=True, stop=False
                        )
                        tensor.matmul(
                            o, lhsA, pw_sb[:, 16:32], start=False, stop=False
                        )
                        mm3 = tensor.matmul(
                            o, lhsB, pw_sb[:, 0:16], start=False, stop=True
                        )
                    mm3.then_inc(mm_sem, 1)

            @block.vector
            def _(vector):
                vector.wait_ge(cs_sem, 32)
                for c in range(nchunk):
                    vector.wait_ge(mm_sem, c + 1)
                    psl = ps[:, c * fw : (c + 1) * fw]
                    vector.tensor_single_scalar(
                        bt[:], psl, 0.0, mybir.AluOpType.is_gt
                    )
                    if gray:
                        vector.tensor_tensor(
                            g[:, 0 : fw - 1],
                            bt[:, 0 : fw - 1],
                            bt[:, 1:fw],
                            op=mybir.AluOpType.not_equal,
                        )
                        vector.tensor_copy(g[:, 7:fw:8], bt[:, 7:fw:8])
                        msrc = g
                    else:
                        msrc = bt
                    vector.tensor_mul(m[:], msrc[:], w_sb[:])
                    vector.tensor_reduce(
                        bf[:],
                        m[:].rearrange("p (k r) -> p k r", r=8),
                        axis=mybir.AxisListType.X,
                        op=mybir.AluOpType.add,
                    )
                    vector.tensor_copy(
                        bi[:, c * fw // 8 : (c + 1) * fw // 8], bf[:]
                    ).then_inc(dve_sem, 1)
    return nc


def _build(gray: bool, pairs: int = PAIRS, chunk_pairs: int = CHUNK_PAIRS):
    nchunk = pairs // chunk_pairs
    u_per_chunk = chunk_pairs // 128
    nc = bass.Bass()
    a_d = nc.declare_dram_parameter("a", [pairs, 128], BF16, isOutput=False)
    b_d = nc.declare_dram_parameter("b", [pairs, 128], BF16, isOutput=False)
    pw_d = nc.declare_dram_parameter("pw", [32, 128], BF16, isOutput=False)
    w_d = nc.declare_dram_parameter("w", [512, 128], BF16, isOutput=False)
    out_d = nc.declare_dram_parameter(
        "out", [nchunk, 2 * chunk_pairs], I32, isOutput=True
    )

    bis = []
    with tile.TileContext(nc) as tc:
        with (
            tc.tile_pool(name="const", bufs=1) as constp,
            tc.tile_pool(name="ab", bufs=8) as abp,
            tc.tile_pool(name="work", bufs=2) as workp,
            tc.tile_pool(name="outp", bufs=8) as outp,
            tc.tile_pool(name="psum", bufs=8, space=bass.MemorySpace.PSUM) as psump,
        ):
            pw_sb = constp.tile([128, 32], BF16)
            nc.sync.dma_start(out=pw_sb[:], in_=pw_d[:], transpose=True)
            w_sb = constp.tile([128, 512], BF16)
            nc.sync.dma_start(out=w_sb[:], in_=w_d[:], transpose=True)
            # Touch w_sb on DVE once so later tensor_tensor ops don't each
            # need a DMA-queue wait (the DVE TT ISA slot allows only one
            # sync wait; walrus rejects two).
            w_touch = constp.tile([128, 1], F32)  # dtype-cast copy is fine
            nc.vector.tensor_copy(w_touch[:], w_sb[:, 0:1])

            for c in range(nchunk):
                sl = slice(c * chunk_pairs, (c + 1) * chunk_pairs)
                a_sb = abp.tile([128, chunk_pairs], BF16, tag="a")
                nc.sync.dma_start(out=a_sb[:], in_=a_d[sl, :], transpose=True)
                b_sb = abp.tile([128, chunk_pairs], BF16, tag="b")
                nc.sync.dma_start(out=b_sb[:], in_=b_d[sl, :], transpose=True)

                fw = 16 * u_per_chunk
                ps = psump.tile([128, fw], F32)
                for u in range(u_per_chunk):
                    lhsA = a_sb[:, u * 128 : (u + 1) * 128]
                    lhsB = b_sb[:, u * 128 : (u + 1) * 128]
                    o = ps[:, u * 16 : (u + 1) * 16]
                    nc.tensor.matmul(o, lhsA, pw_sb[:, 0:16], start=True, stop=False)
                    nc.tensor.matmul(o, lhsA, pw_sb[:, 16:32], start=False, stop=False)
                    nc.tensor.matmul(o, lhsB, pw_sb[:, 0:16], start=False, stop=True)

                bt = workp.tile([128, fw], F32, tag="bt")
                nc.vector.tensor_single_scalar(
                    bt[:], ps[:], 0.0, mybir.AluOpType.is_gt
                )
                if gray:
                    g = workp.tile([128, fw], F32, tag="g")
                    nc.vector.tensor_tensor(
                        g[:, 0 : fw - 1],
                        bt[:, 0 : fw - 1],
                        bt[:, 1:fw],
                        op=mybir.AluOpType.not_equal,
                    )
                    nc.vector.tensor_copy(g[:, 7:fw:8], bt[:, 7:fw:8])
                    msrc = g
                else:
                    msrc = bt
                m = workp.tile([128, fw], F32, tag="m")
                nc.vector.tensor_mul(m[:], msrc[:], w_sb[:, 0:fw])
                bf = workp.tile([128, fw // 8], F32, tag="bf")
                nc.vector.tensor_reduce(
                    bf[:],
                    m[:].rearrange("p (k r) -> p k r", r=8),
                    axis=mybir.AxisListType.X,
                    op=mybir.AluOpType.add,
                )
                bi = outp.tile([128, fw // 8], I32, tag=f"bi{c}")
                nc.vector.tensor_copy(bi[:], bf[:])
                bis.append(bi)
    # Past the TileContext exit: Tile has drained every engine and run an
    # all-engine barrier, so raw output DMAs here need no sync waits at
    # all (the DMA ISA slot only has one wait; inside the context the
    # xbar-transpose serialization would force 2+). Fence completion with
    # one semaphore.
    post = []
    with nc.semaphore("out_sem") as out_sem:
        for c, bi in enumerate(bis):
            h = nc.sync.dma_start(
                out=out_d[c].rearrange("(u p j) -> p u j", p=128, j=2),
                in_=bi[:].rearrange("p (u j) -> p u j", j=2),
            )
            h.then_inc(out_sem, 16)
            post.append(h)
        nc.sync.wait_ge(out_sem, 16 * len(bis))

    # Tile lowered only the instructions it traced; resolve the symbolic
    # tile APs on the raw post-context DMAs the same way tile.py does.
    def _concrete(arg):
        t = arg.bass_ap.tensor
        if hasattr(t, "concrete_tensor"):
            arg.bass_ap.tensor = t.concrete_tensor()
        return arg.bass_ap

    eng = nc.sync
    for h in post:
        inst = h.ins
        inst.ins, inst.outs = eng.lower_symbolic_args(
            inst.ins, inst.outs, _concrete, inst.debug
        )
    return nc


def _prep(mat, proj_dir, perm, enc_vec, gray):
    bf16 = ml_dtypes.bfloat16
    flat = np.ascontiguousarray(mat.reshape(B * H, N, D), dtype=np.float32)
    a_full = flat.astype(bf16)
    b_full = (flat - a_full.astype(np.float32)).astype(bf16)

    p = np.asarray(proj_dir, dtype=np.float32).reshape(D, NPROJ)
    pa = p.astype(bf16)
    pb = (p - pa.astype(np.float32)).astype(bf16)
    pw = np.zeros((128, 32), dtype=bf16)
    pw[0:64, 0:8] = pa
    pw[64:128, 8:16] = pa
    pw[0:64, 16:24] = pb
    pw[64:128, 24:32] = pb
    pw = np.ascontiguousarray(pw.T)  # shipped transposed; X-bar restores

    enc = np.asarray(enc_vec, dtype=np.float32).reshape(NPROJ)
    w = np.tile(enc, 64).reshape(1, 512).repeat(128, axis=0)
    if gray:
        # weights for the XOR-of-adjacent-bits formulation: 2^r for r<7,
        # 128 on the patched r=7 slot -- identical to enc for enc=2^r.
        w = np.tile(
            np.array([1, 2, 4, 8, 16, 32, 64, 128], dtype=np.float32), 64
        ).reshape(1, 512).repeat(128, axis=0)
    w = np.ascontiguousarray(w.T, dtype=np.float32).astype(bf16)

    bh_per_core = B * H // N_CORES
    in_maps = []
    for i in range(N_CORES):
        sh = a_full[i * bh_per_core : (i + 1) * bh_per_core]
        shb = b_full[i * bh_per_core : (i + 1) * bh_per_core]
        in_maps.append(
            {
                "a": np.ascontiguousarray(sh).reshape(PAIRS, 128),
                "b": np.ascontiguousarray(shb).reshape(PAIRS, 128),
                "pw": pw,
                "w": w,
            }
        )
    return in_maps



def _build_v4(gray: bool, pairs: int = PAIRS, chunk_pairs: int = CHUNK_PAIRS):
    """a-only variant of the xbar kernel: halves input DMA (the packet-rate
    bottleneck). Emits per-row min|score| so the host exactly recomputes
    rows inside the bf16 rounding envelope."""
    nchunk = pairs // chunk_pairs
    u_per_chunk = chunk_pairs // 128
    fw = 16 * u_per_chunk
    assert nchunk * fw <= 4096, "psum overflow"
    nc = bass.Bass()
    a_d = nc.declare_dram_parameter("a", [pairs, 128], BF16, isOutput=False)
    pw_d = nc.declare_dram_parameter("pw", [32, 128], BF16, isOutput=False)
    w_d = nc.declare_dram_parameter("w", [fw, 128], BF16, isOutput=False)
    out_d = nc.declare_dram_parameter(
        "out", [nchunk, 2 * chunk_pairs], I32, isOutput=True
    )

    from contextlib import ExitStack

    with ExitStack() as ctx:
        ent = ctx.enter_context
        a_sb = ent(nc.sbuf_tensor("a_sb", [128, pairs], BF16))
        pw_sb = ent(nc.sbuf_tensor("pw_sb", [128, 32], BF16))
        w_sb = ent(nc.sbuf_tensor("w_sb", [128, fw], BF16))
        bt = ent(nc.sbuf_tensor("bt", [128, fw], F32))
        g = ent(nc.sbuf_tensor("g", [128, fw], F32))
        m = ent(nc.sbuf_tensor("m", [128, fw], F32))
        bf = ent(nc.sbuf_tensor("bf", [128, fw // 8], F32))
        flg = ent(nc.sbuf_tensor("flg", [128, fw // 8], F32))
        bi = ent(nc.sbuf_tensor("bi", [128, nchunk * fw // 8], I32))
        ma_sb = ent(nc.sbuf_tensor("ma_sb", [128, nchunk * fw // 8], F32))
        ps = ent(nc.psum_tensor("ps", [128, nchunk * fw], F32))

        cs_sem = ent(nc.semaphore("cs_sem"))
        ch_sems = [ent(nc.semaphore(f"ch_sem{c}")) for c in range(nchunk)]
        mm_sem = ent(nc.semaphore("mm_sem"))
        dve_sem = ent(nc.semaphore("dve_sem"))
        out_sem = ent(nc.semaphore("out_sem"))

        with nc.Block() as block:

            @block.sync
            def _(sync):
                sync.dma_start(
                    out=pw_sb[:], in_=pw_d[:], transpose=True
                ).then_inc(cs_sem, 16)
                sync.dma_start(
                    out=w_sb[:], in_=w_d[:], transpose=True
                ).then_inc(cs_sem, 16)
                for c in range(nchunk):
                    sl = slice(c * chunk_pairs, (c + 1) * chunk_pairs)
                    sync.dma_start(
                        out=a_sb[:, sl], in_=a_d[sl, :], transpose=True
                    ).then_inc(ch_sems[c], 16)
                sync.wait_ge(dve_sem, nchunk)
                for c in range(nchunk):
                    csl = slice(c * fw // 8, (c + 1) * fw // 8)
                    sync.dma_start(
                        out=out_d[c].rearrange("(u p j) -> p u j", p=128, j=2),
                        in_=bi[:, csl].rearrange("p (u j) -> p u j", j=2),
                    ).then_inc(out_sem, 16)
                sync.wait_ge(out_sem, 16 * nchunk)

            @block.tensor
            def _(tensor):
                tensor.wait_ge(cs_sem, 32)
                for c in range(nchunk):
                    tensor.wait_ge(ch_sems[c], 16)
                    for u in range(u_per_chunk):
                        col = c * chunk_pairs + u * 128
                        lhsA = a_sb[:, col : col + 128]
                        o = ps[:, c * fw + u * 16 : c * fw + (u + 1) * 16]
                        tensor.matmul(
                            o, lhsA, pw_sb[:, 0:16], start=True, stop=False
                        )
                        mm = tensor.matmul(
                            o, lhsA, pw_sb[:, 16:32], start=False, stop=True
                        )
                    mm.then_inc(mm_sem, 1)

            @block.vector
            def _(vector):
                vector.wait_ge(cs_sem, 32)
                for c in range(nchunk):
                    vector.wait_ge(mm_sem, c + 1)
                    psl = ps[:, c * fw : (c + 1) * fw]
                    csl = slice(c * fw // 8, (c + 1) * fw // 8)
                    vector.tensor_single_scalar(
                        bt[:], psl, 0.0, mybir.AluOpType.is_gt
                    )
                    if gray:
                        vector.tensor_tensor(
                            g[:, 0 : fw - 1],
                            bt[:, 0 : fw - 1],
                            bt[:, 1:fw],
                            op=mybir.AluOpType.not_equal,
                        )
                        vector.tensor_copy(g[:, 7:fw:8], bt[:, 7:fw:8])
                        msrc = g
                    else:
                        msrc = bt
                    vector.tensor_mul(m[:], msrc[:], w_sb[:])
                    vector.tensor_reduce(
                        bf[:],
                        m[:].rearrange("p (k r) -> p k r", r=8),
                        axis=mybir.AxisListType.X,
                        op=mybir.AluOpType.add,
                    )
                    vector.tensor_reduce(
                        ma_sb[:, csl],
                        psl.rearrange("p (k r) -> p k r", r=8),
                        axis=mybir.AxisListType.X,
                        op=mybir.AluOpType.min,
                        apply_absolute_value=True,
                    )
                    # fuse the "needs host recompute" flag into bit 8 of the
                    # bucket word: out = bucket + 256*(min|score| < TAU)
                    vector.tensor_scalar(
                        flg[:], ma_sb[:, csl], TAU, 256.0,
                        mybir.AluOpType.is_lt, mybir.AluOpType.mult,
                    )
                    vector.tensor_add(bf[:], bf[:], flg[:])
                    vector.tensor_copy(bi[:, csl], bf[:]).then_inc(dve_sem, 1)
    return nc


def _prep_v4(mat, proj_dir, enc_vec, gray):
    bf16 = ml_dtypes.bfloat16
    flat = np.ascontiguousarray(mat.reshape(B * H, N, D), dtype=np.float32)
    a_full = flat.astype(bf16)

    p = np.asarray(proj_dir, dtype=np.float32).reshape(D, NPROJ)
    pa = p.astype(bf16)
    pb = (p - pa.astype(np.float32)).astype(bf16)
    pw = np.zeros((128, 32), dtype=bf16)
    pw[0:64, 0:8] = pa
    pw[64:128, 8:16] = pa
    pw[0:64, 16:24] = pb
    pw[64:128, 24:32] = pb
    pw = np.ascontiguousarray(pw.T)

    wrow = (
        np.array([1, 2, 4, 8, 16, 32, 64, 128], dtype=np.float32)
        if gray
        else np.asarray(enc_vec, dtype=np.float32).reshape(NPROJ)
    )
    w = np.tile(wrow, 64).reshape(1, 512).repeat(128, axis=0)
    w = np.ascontiguousarray(w.T).astype(bf16)

    bh_per_core = B * H // N_CORES
    in_maps = []
    for i in range(N_CORES):
        sh = a_full[i * bh_per_core : (i + 1) * bh_per_core]
        in_maps.append(
            {
                "a": np.ascontiguousarray(sh).reshape(PAIRS, 128),
                "pw": pw,
                "w": w,
            }
        )
    return in_maps


def _build_v5(pairs: int = PAIRS, chunk_pairs: int = CHUNK_PAIRS):
    """Natural-load variant: host ships the bf16 image ALREADY transposed
    ([128, pairs], depth-on-partitions), so every DMA is a plain contiguous
    load at full HBM bandwidth -- no X-bar transpose (measured ~54 GB/s
    effective in v4), no PE transpose. Device always emits raw bin ids
    (enc weights) + a 256*flag bit for rows whose min|score| < TAU; the
    host applies the perm LUT and exactly recomputes flagged rows.
    Output DMAs ride the scalar engine's separate HWDGE ring so they never
    stall the input stream on the sync engine."""
    nchunk = pairs // chunk_pairs
    u_per_chunk = chunk_pairs // 128
    fw = 16 * u_per_chunk  # psum/free width per chunk (512 default)
    assert nchunk * fw <= 4096, "psum overflow"
    nc = bass.Bass()
    a_d = nc.declare_dram_parameter("a", [128, pairs], BF16, isOutput=False)
    pw_d = nc.declare_dram_parameter("pw", [128, 32], BF16, isOutput=False)
    w_d = nc.declare_dram_parameter("w", [128, fw], BF16, isOutput=False)
    out_d = nc.declare_dram_parameter(
        "out", [nchunk, 2 * chunk_pairs], I32, isOutput=True
    )

    from contextlib import ExitStack

    with ExitStack() as ctx:
        ent = ctx.enter_context
        a_sb = ent(nc.sbuf_tensor("a_sb", [128, pairs], BF16))
        pw_sb = ent(nc.sbuf_tensor("pw_sb", [128, 32], BF16))
        w_sb = ent(nc.sbuf_tensor("w_sb", [128, fw], BF16))
        bt = ent(nc.sbuf_tensor("bt", [128, fw], BF16))
        m = ent(nc.sbuf_tensor("m", [128, fw], BF16))
        bf = ent(nc.sbuf_tensor("bf", [128, fw // 8], F32))
        ma = ent(nc.sbuf_tensor("ma", [128, fw // 8], F32))
        flg = ent(nc.sbuf_tensor("flg", [128, fw // 8], F32))
        bi = ent(nc.sbuf_tensor("bi", [128, nchunk * fw // 8], I32))
        ps = ent(nc.psum_tensor("ps", [128, nchunk * fw], F32))

        cs_sem = ent(nc.semaphore("cs_sem"))
        ch_sems = [ent(nc.semaphore(f"ch_sem{c}")) for c in range(nchunk)]
        mm_sem = ent(nc.semaphore("mm_sem"))
        dve_sem = ent(nc.semaphore("dve_sem"))
        out_sem = ent(nc.semaphore("out_sem"))

        with nc.Block() as block:

            @block.sync
            def _(sync):
                sync.dma_start(out=pw_sb[:], in_=pw_d[:]).then_inc(cs_sem, 16)
                sync.dma_start(out=w_sb[:], in_=w_d[:]).then_inc(cs_sem, 16)
                for c in range(nchunk):
                    sl = slice(c * chunk_pairs, (c + 1) * chunk_pairs)
                    sync.dma_start(out=a_sb[:, sl], in_=a_d[:, sl]).then_inc(
                        ch_sems[c], 16
                    )
                sync.wait_ge(out_sem, 16 * nchunk)

            @block.tensor
            def _(tensor):
                tensor.wait_ge(cs_sem, 32)
                for c in range(nchunk):
                    tensor.wait_ge(ch_sems[c], 16)
                    for u in range(u_per_chunk):
                        col = c * chunk_pairs + u * 128
                        lhsA = a_sb[:, col : col + 128]
                        o = ps[:, c * fw + u * 16 : c * fw + (u + 1) * 16]
                        tensor.matmul(
                            o, lhsA, pw_sb[:, 0:16], start=True, stop=False
                        )
                        mm = tensor.matmul(
                            o, lhsA, pw_sb[:, 16:32], start=False, stop=True
                        )
                    mm.then_inc(mm_sem, 1)

            @block.vector
            def _(vector):
                vector.wait_ge(cs_sem, 32)
                for c in range(nchunk):
                    vector.wait_ge(mm_sem, c + 1)
                    psl = ps[:, c * fw : (c + 1) * fw]
                    csl = slice(c * fw // 8, (c + 1) * fw // 8)
                    vector.tensor_single_scalar(
                        bt[:], psl, 0.0, mybir.AluOpType.is_gt
                    )
                    vector.tensor_mul(m[:], bt[:], w_sb[:])
                    vector.tensor_reduce(
                        bf[:],
                        m[:].rearrange("p (k r) -> p k r", r=8),
                        axis=mybir.AxisListType.X,
                        op=mybir.AluOpType.add,
                    )
                    vector.tensor_reduce(
                        ma[:],
                        psl.rearrange("p (k r) -> p k r", r=8),
                        axis=mybir.AxisListType.X,
                        op=mybir.AluOpType.min,
                        apply_absolute_value=True,
                    )
                    vector.tensor_scalar(
                        flg[:], ma[:], TAU, 256.0,
                        mybir.AluOpType.is_lt, mybir.AluOpType.mult,
                    )
                    vector.tensor_add(bf[:], bf[:], flg[:])
                    vector.tensor_copy(bi[:, csl], bf[:]).then_inc(dve_sem, 1)

            @block.scalar
            def _(scalar):
                for c in range(nchunk):
                    csl = slice(c * fw // 8, (c + 1) * fw // 8)
                    scalar.wait_ge(dve_sem, c + 1)
                    scalar.dma_start(
                        out=out_d[c].rearrange("(u p j) -> p u j", p=128, j=2),
                        in_=bi[:, csl].rearrange("p (u j) -> p u j", j=2),
                    ).then_inc(out_sem, 16)
    return nc


def _prep_v5(mat, proj_dir, enc_vec):
    bf16 = ml_dtypes.bfloat16
    flat = np.ascontiguousarray(mat.reshape(B * H, N, D), dtype=np.float32)
    a_full = flat.astype(bf16)

    p = np.asarray(proj_dir, dtype=np.float32).reshape(D, NPROJ)
    pa = p.astype(bf16)
    pb = (p - pa.astype(np.float32)).astype(bf16)
    pw = np.zeros((128, 32), dtype=bf16)
    pw[0:64, 0:8] = pa
    pw[64:128, 8:16] = pa
    pw[0:64, 16:24] = pb
    pw[64:128, 24:32] = pb

    enc = np.asarray(enc_vec, dtype=np.float32).reshape(NPROJ)
    w = np.tile(enc, CHUNK_PAIRS // 128 * 2)
    w = np.broadcast_to(w, (128, w.size)).astype(bf16)

    bh_per_core = B * H // N_CORES
    in_maps = []
    for i in range(N_CORES):
        sh = a_full[i * bh_per_core : (i + 1) * bh_per_core]
        a = sh.reshape(PAIRS, 128)
        in_maps.append(
            {
                "a": np.ascontiguousarray(a.T),
                "pw": pw,
                "w": np.ascontiguousarray(w),
            }
        )
    return in_maps


def _build_v6(pairs: int = PAIRS, chunk_pairs: int = CHUNK_PAIRS):
    """v5 + TensorE col-group rotation + ACT offload.

    v5's PE stream serialized (one LDWEIGHTS per self-loading matmul,
    ~276 ns per 128 pairs). Here each matmul covers M=32 pairs and
    rotates its output across the four 32-partition PE column groups, so
    consecutive (LDWEIGHTS, MATMUL) pairs land in disjoint sub-arrays and
    overlap (the measured 2.38x/3.07x tile-packing concurrency).

    Post-processing: ScalarE evacuates PSUM as p=sign(s-TAU), q=sign(s+TAU)
    (bf16). DVE computes word = sum_r alpha_r*p_r + 128*sum_r q_r + 127.5
    with alpha_r=(2^r-256)/2: unflagged rows give word = bin id in [0,255];
    any score inside (-TAU, TAU] pushes word >= 256 (flag for host redo).
    Single contiguous output DMA at the end (v5's interleaved store made
    8-byte packets and took 73 us)."""
    nchunk = pairs // chunk_pairs
    u_per_chunk = chunk_pairs // 128  # 32
    fw = 16 * u_per_chunk  # 512
    assert nchunk * fw <= 4096, "psum overflow"
    nc = bass.Bass()
    a_d = nc.declare_dram_parameter("a", [128, pairs], BF16, isOutput=False)
    pw_d = nc.declare_dram_parameter("pw", [128, 16], BF16, isOutput=False)
    al_d = nc.declare_dram_parameter("al", [128, fw], BF16, isOutput=False)
    tau_d = nc.declare_dram_parameter("tau", [128, 2], F32, isOutput=False)
    out_d = nc.declare_dram_parameter("out", [128, nchunk * fw // 8], I32, isOutput=True)

    from contextlib import ExitStack

    with ExitStack() as ctx:
        ent = ctx.enter_context
        a_sb = ent(nc.sbuf_tensor("a_sb", [128, pairs], BF16))
        pw_sb = ent(nc.sbuf_tensor("pw_sb", [128, 16], BF16))
        al_sb = ent(nc.sbuf_tensor("al_sb", [128, fw], BF16))
        tau_sb = ent(nc.sbuf_tensor("tau_sb", [128, 2], F32))
        p_w = ent(nc.sbuf_tensor("p_w", [128, 2, fw], BF16))  # double buffer
        q_w = ent(nc.sbuf_tensor("q_w", [128, 2, fw], BF16))
        m1 = ent(nc.sbuf_tensor("m1", [128, fw], BF16))
        bfp = ent(nc.sbuf_tensor("bfp", [128, fw // 8], F32))
        bfq = ent(nc.sbuf_tensor("bfq", [128, fw // 8], F32))
        bi = ent(nc.sbuf_tensor("bi", [128, nchunk * fw // 8], I32))
        ps = ent(nc.psum_tensor("ps", [128, nchunk * fw], F32))

        cs_sem = ent(nc.semaphore("cs_sem"))
        ch_sems = [ent(nc.semaphore(f"ch_sem{c}")) for c in range(nchunk)]
        mm_sem = ent(nc.semaphore("mm_sem"))
        act_sem = ent(nc.semaphore("act_sem"))
        dve_sem = ent(nc.semaphore("dve_sem"))
        out_sem = ent(nc.semaphore("out_sem"))

        with nc.Block() as block:

            @block.sync
            def _(sync):
                sync.dma_start(out=pw_sb[:], in_=pw_d[:]).then_inc(cs_sem, 16)
                sync.dma_start(out=al_sb[:], in_=al_d[:]).then_inc(cs_sem, 16)
                sync.dma_start(out=tau_sb[:], in_=tau_d[:]).then_inc(cs_sem, 16)
                for c in range(nchunk):
                    sl = slice(c * chunk_pairs, (c + 1) * chunk_pairs)
                    sync.dma_start(out=a_sb[:, sl], in_=a_d[:, sl]).then_inc(
                        ch_sems[c], 16
                    )
                sync.wait_ge(out_sem, 16)

            @block.tensor
            def _(tensor):
                tensor.wait_ge(cs_sem, 48)
                for c in range(nchunk):
                    tensor.wait_ge(ch_sems[c], 16)
                    for t in range(4 * u_per_chunk):
                        g, u2 = t % 4, t // 4
                        col = c * chunk_pairs + t * 32
                        o = ps[32 * g : 32 * g + 32,
                              c * fw + u2 * 16 : c * fw + (u2 + 1) * 16]
                        mm = tensor.matmul(
                            o, a_sb[:, col : col + 32], pw_sb[:],
                            start=True, stop=True, tile_position=(0, 32 * g),
                        )
                    mm.then_inc(mm_sem, 1)

            @block.scalar
            def _(scalar):
                scalar.wait_ge(cs_sem, 48)
                for c in range(nchunk):
                    scalar.wait_ge(mm_sem, c + 1)
                    if c >= 2:
                        scalar.wait_ge(dve_sem, c - 1)  # buffer slot reuse
                    psl = ps[:, c * fw : (c + 1) * fw]
                    scalar.activation(
                        p_w[:, c % 2, :], psl,
                        mybir.ActivationFunctionType.Sign, bias=tau_sb[:, 0:1],
                    )
                    scalar.activation(
                        q_w[:, c % 2, :], psl,
                        mybir.ActivationFunctionType.Sign, bias=tau_sb[:, 1:2],
                    ).then_inc(act_sem, 1)
                scalar.wait_ge(dve_sem, nchunk)
                scalar.dma_start(out=out_d[:], in_=bi[:]).then_inc(out_sem, 16)

            @block.vector
            def _(vector):
                vector.wait_ge(cs_sem, 48)
                for c in range(nchunk):
                    vector.wait_ge(act_sem, c + 1)
                    csl = slice(c * fw // 8, (c + 1) * fw // 8)
                    vector.tensor_mul(m1[:], p_w[:, c % 2, :], al_sb[:])
                    vector.tensor_reduce(
                        bfp[:],
                        m1[:].rearrange("p (k r) -> p k r", r=8),
                        axis=mybir.AxisListType.X,
                        op=mybir.AluOpType.add,
                    )
                    vector.tensor_reduce(
                        bfq[:],
                        q_w[:, c % 2, :].rearrange("p (k r) -> p k r", r=8),
                        axis=mybir.AxisListType.X,
                        op=mybir.AluOpType.add,
                    )
                    vector.tensor_scalar(
                        bfq[:], bfq[:], 128.0, 127.5,
                        mybir.AluOpType.mult, mybir.AluOpType.add,
                    )
                    vector.tensor_tensor(
                        bi[:, csl], bfp[:], bfq[:], op=mybir.AluOpType.add
                    ).then_inc(dve_sem, 1)
    return nc


def _prep_v6(mat, proj_dir):
    bf16 = ml_dtypes.bfloat16
    flat = np.ascontiguousarray(mat.reshape(B * H, N, D), dtype=np.float32)
    a_full = flat.astype(bf16)

    p = np.asarray(proj_dir, dtype=np.float32).reshape(D, NPROJ)
    pa = p.astype(bf16)
    pw = np.zeros((128, 16), dtype=bf16)
    pw[0:64, 0:8] = pa
    pw[64:128, 8:16] = pa

    alpha = (2.0 ** np.arange(NPROJ, dtype=np.float32) - 256.0) / 2.0
    al = np.tile(alpha, CHUNK_PAIRS // 128 * 2)
    al = np.broadcast_to(al, (128, al.size)).astype(bf16)

    tau = np.empty((128, 2), dtype=np.float32)
    tau[:, 0] = -TAU
    tau[:, 1] = TAU

    bh_per_core = B * H // N_CORES
    in_maps = []
    for i in range(N_CORES):
        sh = a_full[i * bh_per_core : (i + 1) * bh_per_core]
        a = sh.reshape(PAIRS, 128)
        in_maps.append(
            {
                "a": np.ascontiguousarray(a.T),
                "pw": pw,
                "al": np.ascontiguousarray(al),
                "tau": tau,
            }
        )
    return in_maps


def _decode_v6(dev_out):
    """[128, 512] device words -> [65536] per-core row-ordered words.

    Device word at (partition 32g+p2, col c*64+u2*2+j) belongs to row
    2*(c*4096 + u2*128 + g*32 + p2) + j of the core shard."""
    w = dev_out.reshape(4, 32, NCHUNK, U_PER_CHUNK, 2)  # (g, p2, c, u2, j)
    return np.ascontiguousarray(w.transpose(2, 3, 0, 1, 4)).reshape(-1)


def _build_v7(pairs: int = PAIRS, chunk_pairs: int = CHUNK_PAIRS):
    """Streaming design: pw stationary-ish, `a` is the MOVING operand.

    v6's floor was ~256 x (LDWEIGHTS + isolated-MM latency) ~ 45 us: with
    `a` as the stationary operand every 32 pairs costs a weight load plus a
    ~178 ns matmul latency, and sub-array rotation caps concurrency at 4.
    Here each score matmul streams 512 pairs (N=512) through a tiny
    [128, 32] weight block [pw | -pw], rotating output col groups 0..3, so
    a 4-tile "supergroup" fills psum [128, 512] with rows 32g+(0:16) = s,
    32g+(16:32) = -s. One ACT Sign pass (bias -TAU) turns that into
    p = sign(s-TAU) (rows 0:16) and -q = -sign(s+TAU) (rows 16:32) as
    bf16 in SBUF. A second block-diagonal matmul wvec4 [128, 8]
    (alpha_r = (2^r-256)/2 on p-rows, -128 on -q-rows, columns 2i+j)
    collapses K=128 -> words for all 4 tiles at once: psum [8@32g, 512],
    word = bin + 256*gapcount - 127.5. DVE adds 127.5 and casts to i32;
    one full-width output DMA at the end. ~84 PE instructions total."""
    nchunk = pairs // chunk_pairs
    ngroup = pairs // 2048  # 4-tile supergroups of 2048 pairs
    assert ngroup == 16 and chunk_pairs == 4096
    nc = bass.Bass()
    a_d = nc.declare_dram_parameter("a", [128, pairs], BF16, isOutput=False)
    cst_d = nc.declare_dram_parameter("cst", [128, 44], BF16, isOutput=False)
    I16 = mybir.dt.int16
    out_d = nc.declare_dram_parameter("out", [4, 8, 2048], I16, isOutput=True)

    from contextlib import ExitStack

    with ExitStack() as ctx:
        ent = ctx.enter_context
        a_sb = ent(nc.sbuf_tensor("a_sb", [128, pairs], BF16))
        cst_sb = ent(nc.sbuf_tensor("cst_sb", [128, 44], BF16))
        pw_sb = cst_sb[:, 0:32]
        wv_sb = cst_sb[:, 32:40]
        tau_sb = cst_sb[:, 40:42].bitcast(F32)
        bits = ent(nc.sbuf_tensor("bits", [128, 2, 512], BF16))  # double buffer
        bi = ent(nc.sbuf_tensor("bi", [128, 2048], I16))
        # psum: score slots 0-2 at free [0:1536); words at free [2048:4096)
        ps = ent(nc.psum_tensor("ps", [128, 4096], F32))

        cs_sem = ent(nc.semaphore("cs_sem"))
        ch_sems = [ent(nc.semaphore(f"ch_sem{c}")) for c in range(nchunk)]
        mm_sem = ent(nc.semaphore("mm_sem"))
        act_sem = ent(nc.semaphore("act_sem"))
        wrd_sem = ent(nc.semaphore("wrd_sem"))
        dve_sem = ent(nc.semaphore("dve_sem"))
        out_sem = ent(nc.semaphore("out_sem"))

        def score_group(tensor, G):
            slot = G % 3
            for g in range(4):
                t = 4 * G + g
                mm = tensor.matmul(
                    ps[32 * g : 32 * (g + 1), 512 * slot : 512 * (slot + 1)],
                    pw_sb,
                    a_sb[:, 512 * t : 512 * (t + 1)],
                    start=True, stop=True, tile_position=(0, 32 * g),
                )
            mm.then_inc(mm_sem, 1)

        def word_mm(tensor, G):
            g, s = G % 4, G // 4
            tensor.matmul(
                ps[32 * g : 32 * g + 8, 2048 + 512 * s : 2048 + 512 * (s + 1)],
                wv_sb,
                bits[:, G % 2, :],
                start=True, stop=True, tile_position=(0, 32 * g),
            ).then_inc(wrd_sem, 1)

        with nc.Block() as block:

            # inputs as 8x1MB transfers (2 supergroups each) interleaved
            # across the two HWDGE rings -- 1MB units showed no ramp and
            # two rings aggregate to ~358 GB/s. ch_sems[k] gates G=2k,2k+1.
            def a_dma(eng, k):
                sl = slice(4096 * k, 4096 * (k + 1))
                eng.dma_start(out=a_sb[:, sl], in_=a_d[:, sl]).then_inc(
                    ch_sems[k], 16
                )

            # output piece (g, s) = words of supergroup G=4s+g, 8 KB,
            # ready after evac G; all but the last two ride the idle sync
            # ring and overlap the stream
            def out_piece(eng, g, s):
                eng.wait_ge(dve_sem, 4 * s + g + 1)
                eng.dma_start(
                    out=out_d[g][:, 512 * s : 512 * (s + 1)],
                    in_=bi[32 * g : 32 * g + 8, 512 * s : 512 * (s + 1)],
                ).then_inc(out_sem, 16)

            @block.sync
            def _(sync):
                sync.dma_start(out=cst_sb[:], in_=cst_d[:]).then_inc(cs_sem, 16)
                for k in range(0, nchunk, 2):
                    a_dma(sync, k)
                for s in range(4):
                    for g in range(4):
                        if (g, s) in ((1, 3), (3, 3)):
                            continue
                        out_piece(sync, g, s)
                sync.wait_ge(out_sem, 256)

            @block.tensor
            def _(tensor):
                tensor.wait_ge(cs_sem, 16)
                for G in range(ngroup):
                    if G % 2 == 0:
                        tensor.wait_ge(ch_sems[G // 2], 16)
                    if G >= 3:
                        tensor.wait_ge(act_sem, G - 2)  # psum slot G%3 free
                    score_group(tensor, G)
                    if G >= 1:
                        tensor.wait_ge(act_sem, G)  # bits(G-1) ready
                        word_mm(tensor, G - 1)
                tensor.wait_ge(act_sem, ngroup)
                word_mm(tensor, ngroup - 1)

            @block.scalar
            def _(scalar):
                for k in range(1, nchunk, 2):
                    a_dma(scalar, k)
                for G in range(ngroup):
                    scalar.wait_ge(mm_sem, G + 1)
                    if G >= 2:
                        scalar.wait_ge(wrd_sem, G - 1)  # bits buf G%2 free
                    slot = G % 3
                    scalar.activation(
                        bits[:, G % 2, :],
                        ps[:, 512 * slot : 512 * (slot + 1)],
                        mybir.ActivationFunctionType.Sign,
                        bias=tau_sb,
                    ).then_inc(act_sem, 1)
                out_piece(scalar, 1, 3)
                out_piece(scalar, 3, 3)

            @block.vector
            def _(vector):
                for G in range(ngroup):
                    vector.wait_ge(wrd_sem, G + 1)
                    g, s = G % 4, G // 4
                    vector.tensor_scalar_add(
                        bi[32 * g : 32 * g + 8, 512 * s : 512 * (s + 1)],
                        ps[32 * g : 32 * g + 8, 2048 + 512 * s : 2048 + 512 * (s + 1)],
                        127.5,
                    ).then_inc(dve_sem, 1)
    return nc


def _prep_v7(mat, proj_dir):
    bf16 = ml_dtypes.bfloat16
    flat = np.ascontiguousarray(mat.reshape(B * H, N, D), dtype=np.float32)
    a_full = flat.astype(bf16)

    p = np.asarray(proj_dir, dtype=np.float32).reshape(D, NPROJ)
    pa = p.astype(bf16)
    pw = np.zeros((128, 32), dtype=bf16)
    pw[0:64, 0:8] = pa
    pw[64:128, 8:16] = pa
    pw[:, 16:32] = -pw[:, 0:16]

    alpha = (2.0 ** np.arange(NPROJ, dtype=np.float32) - 256.0) / 2.0
    wv = np.zeros((128, 8), dtype=np.float32)
    for i in range(4):
        for j in range(2):
            for r in range(NPROJ):
                wv[32 * i + 8 * j + r, 2 * i + j] = alpha[r]
                wv[32 * i + 16 + 8 * j + r, 2 * i + j] = -128.0
    wv = wv.astype(bf16)

    cst = np.zeros((128, 44), dtype=bf16)
    cst[:, 0:32] = pw
    cst[:, 32:40] = wv
    cst[:, 40:42] = np.full((128, 1), -TAU, dtype=np.float32).view(bf16)

    bh_per_core = B * H // N_CORES
    in_maps = []
    for i in range(N_CORES):
        sh = a_full[i * bh_per_core : (i + 1) * bh_per_core]
        a = sh.reshape(PAIRS, 128)
        aT = np.ascontiguousarray(a.T)  # [128, PAIRS]
        in_maps.append({"a": aT, "cst": cst})
    return in_maps


def _decode_v7(dev_out):
    """[4, 8, 2048] device words -> [65536] per-core row-ordered words.

    Word of tile 16s+4g+i, pair tile*512+n, parity j sits at
    dev[g, 2i+j, 512s + n]."""
    v = dev_out.reshape(4, 4, 2, 4, 512)               # (g, i, j, s, n)
    return np.ascontiguousarray(v.transpose(3, 0, 1, 4, 2)).reshape(-1)


def _is_gray_setup(perm, enc_vec):
    perm = np.asarray(perm).reshape(-1)
    enc = np.asarray(enc_vec).reshape(-1)
    if perm.shape[0] != 256 or enc.shape[0] != NPROJ:
        return False
    idx = np.arange(256, dtype=np.int64)
    return bool(
        np.array_equal(perm, idx ^ (idx >> 1)) and np.array_equal(enc, 2 ** np.arange(8))
    )


def kernel(mat, proj_dir, perm, enc_vec, _trace=False, _tmpdir=None):
    enc = np.asarray(enc_vec).reshape(-1).astype(np.int64)
    perm_arr = np.asarray(perm).reshape(-1).astype(np.int64)
    std_enc = enc.shape[0] == NPROJ and np.array_equal(enc, 2 ** np.arange(NPROJ))
    if not (std_enc and perm_arr.shape[0] == 256):
        # Pathological setup the device word-packing doesn't cover (the
        # harness never hits this): plain host computation.
        flat = np.ascontiguousarray(mat.reshape(B * H * N, D), dtype=np.float64)
        p = np.asarray(proj_dir, dtype=np.float64).reshape(D, NPROJ)
        bits = (flat @ p > 0).astype(np.int64)
        bins = (bits * enc).sum(-1)
        out = perm_arr[bins].reshape(B, H, N).astype(np.int32)
        return (out, None) if _trace else out

    if "v7" not in _cache:
        _cache["v7"] = _build_v7()
    nc = _cache["v7"]

    in_maps = _prep_v7(mat, proj_dir)
    res = run_bass_kernel_spmd(
        nc, in_maps, core_ids=list(range(N_CORES)), trace=_trace, tmpdir=_tmpdir
    )
    word = np.concatenate(
        [_decode_v7(np.asarray(r["out"])) for r in res.results]
    ).astype(np.int64)
    buckets = perm_arr[word & 255]  # device emits raw bin ids
    flagged = word >= 256           # device min|score| < TAU

    # Host fix-up: rows whose smallest |bf16 score| is inside the rounding
    # envelope get recomputed exactly.
    idx = np.nonzero(flagged)[0]
    if idx.size:
        flat = np.ascontiguousarray(mat.reshape(B * H * N, D), dtype=np.float32)
        p = np.asarray(proj_dir, dtype=np.float32).reshape(D, NPROJ)
        sc = flat[idx] @ p
        bits = (sc > 0).astype(np.int64)
        bins = (bits * enc).sum(-1)
        buckets[idx] = perm_arr[bins]
    out = buckets.reshape(B, H, N).astype(np.int32)
    if _trace:
        return out, res
    return out

